# revision 13
# baseline (speedup 1.0000x reference)
"""Multi-head causal attention (B=1, S=4096, D=768, H=12) on 8 trn2 NeuronCores.

Sharding: tensor-parallel over heads + causal-balanced split of the query range.
  - cores 0-5 ("late"):  2 heads each, q in [1792, 4096), k in [0, 4096)
  - cores 6-7 ("early"): 6 heads each, q in [0, 1792),  k in [0, 1792)

v2 design (vs baseline):
  - bf16 matmul operands everywhere (x, W, q, k, v, probs, Wo). Enables FWL
    (2x faster LDWEIGHTS) which unlocks true row-tiled concurrency of the two
    heads' score matmuls (head A on PE rows 0-63, head B on rows 64-127).
  - projection chunks (256 seq) interleaved with attention qtiles so the PE
    never idles long enough for the HAM clock-gate to re-throttle, and the
    scalar engine's exp stream (the real wall, ~0.85ns/elem/lane) overlaps
    the projection phase.
  - one exp instruction per (2-ktile x 2-head) group: [128, 1024] free.
  - both heads' flash accumulators packed into ONE psum bank [65 used, 512]
    (start=True only on the first matmul: the bank-wide has_written clear
    makes head B's first accumulate an overwrite; stop=True only on the
    final matmul).  Out-projection psum shares the same pool tag.
  - normalization: reciprocal_approx_fast + gpsimd partition_broadcast.
  - y written as bf16 partials, one DMA per qtile (host sums in fp32).

All inputs are taken at full shape; slicing/transposition happens on host.
"""

import sys
import threading

sys.path.insert(0, "/opt/trn_rl_repo")

import numpy as np
import ml_dtypes

import concourse.bass as bass
import concourse.mybir as mybir
import concourse.tile as tile
from concourse import bacc
from concourse.masks import make_identity

# ---------------------------------------------------------------- constants
B, S, D, H, DH = 1, 4096, 768, 12, 64
SCALE = DH ** -0.5
P = 128          # sbuf partitions
QT = 256         # query tile (free axis of scores)
KT = 128         # key tile (partition axis of scores)
CK = 256         # projection chunk (seq)
XW = 512         # x dma tile width (2 chunks)
GMAX = 3         # max ktiles per score/exp group
SPLIT = 1792     # early/late query split point
DT = mybir.dt.float32
BF = mybir.dt.bfloat16

CLASSES = {
    # name: (n_pairs, q0, q1, k_len)
    "late": (1, SPLIT, S, S),
    "early": (3, 0, SPLIT, SPLIT),
}


def _groups(n):
    """Split n (even) non-diagonal ktiles into chunks of 3 and 2."""
    out = []
    while n >= 5 or n == 3:
        out.append(3)
        n -= 3
    while n > 0:
        out.append(2)
        n -= 2
    return out


def build_module(cls):
    n_pairs, q0, q1, k_len = CLASSES[cls]
    f_c = P * n_pairs            # per-core feature width of each projection
    q_len = q1 - q0
    n_ck = k_len // CK           # projection chunks
    n_kt = k_len // KT           # ktiles of the core's k-support
    n_qt = q_len // QT           # qtiles of the core's q-range
    n_dt = D // P                # 6 contraction tiles for the projections
    c_q0 = q0 // CK              # first chunk whose q-projection is needed
    pre = (q0 + QT) // CK        # chunks needed before qtile 0 can run

    nc = bacc.Bacc("TRN2", target_bir_lowering=False, debug=False,
                   enable_asserts=True, num_devices=1)

    xT = nc.dram_tensor("xT", [D, k_len], BF, kind="ExternalInput")
    wqT = nc.dram_tensor("wqT", [D, f_c], BF, kind="ExternalInput")
    wkT = nc.dram_tensor("wkT", [D, f_c], BF, kind="ExternalInput")
    wvT = nc.dram_tensor("wvT", [D, f_c], BF, kind="ExternalInput")
    bq = nc.dram_tensor("bq", [f_c, 1], DT, kind="ExternalInput")
    bv = nc.dram_tensor("bv", [f_c, 1], DT, kind="ExternalInput")
    woT = nc.dram_tensor("woT", [f_c, D], BF, kind="ExternalInput")
    dmask = nc.dram_tensor("dmask", [P, 2 * QT], BF, kind="ExternalInput")
    yT = nc.dram_tensor("yT", [D, q_len], BF, kind="ExternalOutput")

    with tile.TileContext(nc) as tc:
        with (
            tc.tile_pool(name="w", bufs=1) as sb_w,
            tc.tile_pool(name="x", bufs=2) as sb_x,
            tc.tile_pool(name="persist", bufs=1) as sb_per,
            tc.tile_pool(name="vt", bufs=2) as sb_vt,
            tc.tile_pool(name="exp", bufs=3) as sb_exp,
            tc.tile_pool(name="aTp", bufs=2) as sb_a,
            tc.tile_pool(name="rn", bufs=3) as sb_rn,
            tc.tile_pool(name="yout", bufs=2) as sb_y,
            tc.tile_pool(name="big", bufs=2, space="PSUM") as ps_big,
            tc.tile_pool(name="av", bufs=1, space="PSUM") as ps_av,
        ):
            # ---------------- constants / weights to SBUF
            wq_sb = sb_w.tile([P, n_dt, f_c], BF, tag="wq")
            nc.sync.dma_start(out=wq_sb, in_=wqT.rearrange("(t p) f -> p t f", p=P))
            wk_sb = sb_w.tile([P, n_dt, f_c], BF, tag="wk")
            nc.sync.dma_start(out=wk_sb, in_=wkT.rearrange("(t p) f -> p t f", p=P))
            wv_sb = sb_w.tile([P, n_dt, f_c], BF, tag="wv")
            nc.sync.dma_start(out=wv_sb, in_=wvT.rearrange("(t p) f -> p t f", p=P))
            bq_sb = sb_w.tile([P, n_pairs], DT, tag="bq")
            nc.sync.dma_start(out=bq_sb, in_=bq.rearrange("(n p) o -> p (n o)", p=P))
            bv_sb = sb_w.tile([P, n_pairs], DT, tag="bv")
            nc.sync.dma_start(out=bv_sb, in_=bv.rearrange("(n p) o -> p (n o)", p=P))
            wo_sb = sb_w.tile([P, n_pairs, n_dt, P], BF, tag="wo")
            nc.sync.dma_start(
                out=wo_sb,
                in_=woT.rearrange("(n p) (t m) -> p n t m", p=P, m=P))
            dmask_sb = sb_w.tile([P, 2, QT], BF, tag="dmask")
            nc.sync.dma_start(
                out=dmask_sb, in_=dmask.rearrange("p (a q) -> p a q", a=2))
            ident_f = sb_w.tile([P, P], DT, tag="ident_f")
            make_identity(nc, ident_f)
            ident = sb_w.tile([P, P], BF, tag="ident")
            nc.vector.tensor_copy(ident, ident_f)

            # ---------------- persistent activations (head pair packed on
            # partitions: head A rows 0-63, head B rows 64-127)
            qT = [sb_per.tile([P, q_len], BF, tag=f"qT{p}", name=f"qT{p}")
                  for p in range(n_pairs)]
            kT = [sb_per.tile([P, k_len], BF, tag=f"kT{p}", name=f"kT{p}")
                  for p in range(n_pairs)]
            # per ktile: [V_A(64) | 1 | pad | V_B(64) | 1 | pad], k on partitions
            vkt = [sb_per.tile([P, n_kt, 132], BF, tag=f"vk{p}", name=f"vk{p}")
                   for p in range(n_pairs)]
            for p in range(n_pairs):
                nc.vector.memset(vkt[p][:, :, 64:65], 1.0)
                nc.vector.memset(vkt[p][:, :, 130:131], 1.0)

            # ---------------- projection chunk (CK=256 seq positions)
            cur_xts = []

            def emit_chunk(c):
                s0 = c * CK
                if c % 2 == 0:  # dma covers chunks c, c+1
                    w = min(XW, k_len - s0)
                    cur_xts.clear()
                    for dti in range(n_dt):
                        xt = sb_x.tile([P, XW], BF, tag=f"xt{dti}")
                        nc.sync.dma_start(
                            out=xt[:, :w],
                            in_=xT[dti * P:(dti + 1) * P, s0:s0 + w])
                        cur_xts.append(xt)
                xts = cur_xts
                xo = (c % 2) * CK
                do_q = c >= c_q0
                for p in range(n_pairs):
                    ps = ps_big.tile([P, 3, CK], DT, tag="big", name="ps_prj")
                    # each chain fully start->stop before the next begins: a
                    # start=True clears has_written for its WHOLE psum bank,
                    # so chains sharing a bank must not interleave
                    w_all = {0: wq_sb, 1: wk_sb, 2: wv_sb}
                    for i in ((0, 1, 2) if do_q else (1, 2)):
                        for dti in range(n_dt):
                            nc.tensor.matmul(
                                ps[:, i, :],
                                w_all[i][:, dti, p * P:(p + 1) * P],
                                xts[dti][:, xo:xo + CK],
                                start=dti == 0, stop=dti == n_dt - 1)
                    # k/v staging: scalar engine for the 3-pair class (its
                    # vector engine is busier), vector for the 1-pair class
                    # (its scalar engine is the exp wall)
                    if n_pairs > 1:
                        nc.scalar.copy(kT[p][:, s0:s0 + CK], ps[:, 1, :])
                    else:
                        nc.vector.tensor_copy(kT[p][:, s0:s0 + CK], ps[:, 1, :])
                    if do_q:
                        nc.vector.tensor_scalar_add(
                            qT[p][:, s0 - q0:s0 - q0 + CK],
                            ps[:, 0, :], bq_sb[:, p:p + 1])
                    vt = sb_vt.tile([P, CK], BF, tag=f"vt{p}")
                    if n_pairs > 1:
                        nc.scalar.activation(
                            vt, ps[:, 2, :],
                            mybir.ActivationFunctionType.Identity,
                            bias=bv_sb[:, p:p + 1])
                    else:
                        nc.vector.tensor_scalar_add(
                            vt, ps[:, 2, :], bv_sb[:, p:p + 1])
                    # transpose each 128-wide ktile into vkt layout
                    for j in range(CK // KT):
                        kt_i = (s0 // KT) + j
                        pt = ps_big.tile([P, P], BF, tag="big", name="pt")
                        nc.tensor.transpose(
                            pt, vt[:, j * KT:(j + 1) * KT], ident)
                        dst = vkt[p][:, kt_i, :].rearrange(
                            "p (h c) -> p h c", h=2)[:, :, 0:64]
                        nc.vector.tensor_copy(
                            dst, pt.rearrange("p (h c) -> p h c", h=2))

            # ---------------- attention qtile
            def emit_qtile(qt):
                g = q0 // QT + qt
                n_kt_q = 2 * g + 2
                plan = [(c, False) for c in _groups(n_kt_q - 2)] + [(2, True)]
                a_tiles = []
                for p in range(n_pairs):
                    # av: [P, (head, half, QT)] -- each head's accumulation
                    # group gets its own psum bank (numerator rows 0-63 +
                    # denominator row 64 in the first half); the second half
                    # of each bank doubles as out-projection psum
                    av = ps_av.tile([P, 2, 2, QT], DT, tag="av", name="av")
                    qh = [qT[p][hi * 64:(hi + 1) * 64,
                                qt * QT:(qt + 1) * QT] for hi in (0, 1)]
                    kt0 = 0
                    for (gsz, diag) in plan:
                        kts = list(range(kt0, kt0 + gsz))
                        kt0 += gsz
                        ps_sc = ps_big.tile([P, 2, GMAX, QT], DT, tag="big",
                                            name="ps_sc")
                        for j, k in enumerate(kts):
                            for hi in (0, 1):
                                nc.tensor.matmul(
                                    ps_sc[:, hi, j, :],
                                    kT[p][hi * 64:(hi + 1) * 64,
                                          k * KT:(k + 1) * KT],
                                    qh[hi], start=True, stop=True)
                        ex = sb_exp.tile([P, 2, GMAX, QT], BF, tag="ex")
                        nc.scalar.activation(
                            ex[:, :, 0:gsz, :], ps_sc[:, :, 0:gsz, :],
                            mybir.ActivationFunctionType.Exp, scale=SCALE)
                        if diag:
                            for hi in (0, 1):
                                eng = nc.gpsimd if n_pairs > 1 else nc.vector
                                eng.tensor_mul(
                                    ex[:, hi, 0:2, :], ex[:, hi, 0:2, :],
                                    dmask_sb)
                        for j, k in enumerate(kts):
                            for hi in (0, 1):
                                nc.tensor.matmul(
                                    av[0:65, hi, 0, :],
                                    vkt[p][:, k, 66 * hi:66 * hi + 65],
                                    ex[:, hi, j, :],
                                    start=(k == 0),
                                    stop=(k == n_kt_q - 1))
                    # normalize: a = num * (1/den)
                    aT = sb_a.tile([P, QT], BF, tag=f"aT{p}")
                    # dens: psum row 64 of each head's bank -> one sbuf row;
                    # broadcast raw dens to all partitions (gpsimd), then a
                    # full-tile reciprocal_approx_fast (the custom DVE op is
                    # only correct on [128, N] base-0 tiles on HW)
                    dd = sb_rn.tile([1, 2, QT], DT, tag="dd")
                    nc.vector.tensor_copy(dd, av[64:65, :, 0, :])
                    db = sb_rn.tile([P, 2, QT], DT, tag="db")
                    nc.gpsimd.partition_broadcast(
                        db.rearrange("p h q -> p (h q)"),
                        dd.rearrange("p h q -> p (h q)"))
                    rb = sb_rn.tile([P, 2, QT], DT, tag="rb")
                    nc.vector.reciprocal_approx_fast(rb, db)
                    for hi in (0, 1):
                        nc.vector.tensor_mul(
                            aT[hi * 64:(hi + 1) * 64, :],
                            av[0:64, hi, 0, :], rb[hi * 64:hi * 64 + 64, hi, :])
                    a_tiles.append(aT)
                # out-projection (psum shares the "av" pool tag)
                ysb = sb_y.tile([P, n_dt, QT], BF, tag="y")
                for mt in range(n_dt):
                    ps_y = av[:, mt % 2, 1, :]
                    for p in range(n_pairs):
                        nc.tensor.matmul(
                            ps_y, wo_sb[:, p, mt, :], a_tiles[p],
                            start=(p == 0), stop=(p == n_pairs - 1))
                    nc.vector.tensor_copy(ysb[:, mt, :], ps_y)
                nc.sync.dma_start(
                    out=yT.rearrange("(t p) q -> p t q", p=P)[
                        :, :, qt * QT:(qt + 1) * QT],
                    in_=ysb)

            # ---------------- schedule: prefix chunks, then interleave
            for c in range(pre):
                emit_chunk(c)
            for qt in range(n_qt):
                emit_qtile(qt)
                if pre + qt < n_ck:
                    emit_chunk(pre + qt)

    nc.compile()
    return nc


# ---------------------------------------------------------------- host side
def _head_cols(heads):
    """column indices into a [*, 768] head-blocked axis for the given heads"""
    return np.concatenate([np.arange(h * DH, (h + 1) * DH) for h in heads])


def make_in_maps(x, W_in, b_in, W_out):
    """Returns (late_in_maps[6], early_in_maps[2])."""
    xT = np.ascontiguousarray(x.reshape(S, D).T).astype(ml_dtypes.bfloat16)
    WT = np.ascontiguousarray(W_in.T)                     # [768, 2304]
    WoT = np.ascontiguousarray(W_out.T)                   # [768, 768]

    tri = np.triu(np.ones((P, P), np.float32))            # k <= q
    dm = np.zeros((P, 2 * QT), np.float32)
    dm[:, 0:128] = tri          # diag ktile j=0: [tri | ones]
    dm[:, 128:256] = 1.0
    dm[:, 384:512] = tri        # diag ktile j=1: [zeros | tri]
    dm = dm.astype(ml_dtypes.bfloat16)

    def core_inputs(heads, cls):
        _, q0, q1, k_len = CLASSES[cls]
        cols = _head_cols(heads)
        bf = ml_dtypes.bfloat16
        wq = np.ascontiguousarray(WT[:, cols]).astype(bf)
        wk = np.ascontiguousarray(WT[:, 768 + cols]).astype(bf)
        wv = np.ascontiguousarray(WT[:, 1536 + cols]).astype(bf)
        bqc = np.ascontiguousarray(b_in[cols][:, None]).astype(np.float32)
        bvc = np.ascontiguousarray(
            b_in[1536 + cols][:, None]).astype(np.float32)
        wo = np.ascontiguousarray(WoT[cols, :]).astype(bf)
        return {
            "xT": np.ascontiguousarray(xT[:, :k_len]),
            "wqT": wq, "wkT": wk, "wvT": wv,
            "bq": bqc, "bv": bvc, "woT": wo, "dmask": dm,
        }

    late = [core_inputs([2 * c, 2 * c + 1], "late") for c in range(6)]
    early = [core_inputs(list(range(6 * e, 6 * e + 6)), "early")
             for e in range(2)]
    return late, early


def assemble_output(late_res, early_res, b_out):
    yT = np.zeros((D, S), np.float32)
    for r in late_res:
        yT[:, SPLIT:] += np.asarray(r["yT"], dtype=np.float32)
    for r in early_res:
        yT[:, :SPLIT] += np.asarray(r["yT"], dtype=np.float32)
    y = yT.T + b_out[None, :]
    return y.reshape(B, S, D).astype(np.float32)


# ------------------------------------------------- pjrt runner (explicit devices)
def _run_group(nc, in_maps, devices):
    """run_bass_via_pjrt equivalent on an explicit device subset."""
    import jax
    from jax.sharding import Mesh, PartitionSpec
    from jax.experimental.shard_map import shard_map
    from concourse import bass2jax
    from concourse.bass2jax import _bass_exec_p, partition_id_tensor

    bass2jax.install_neuronx_cc_hook()
    n_cores = len(in_maps)
    partition_name = (nc.partition_id_tensor.name
                      if nc.partition_id_tensor else None)

    in_names, out_names, out_avals, zero_outs = [], [], [], []
    for alloc in nc.m.functions[0].allocations:
        if not isinstance(alloc, mybir.MemoryLocationSet):
            continue
        name = alloc.memorylocations[0].name
        if alloc.kind == "ExternalInput":
            if name != partition_name:
                in_names.append(name)
        elif alloc.kind == "ExternalOutput":
            shape = tuple(alloc.tensor_shape)
            dtype = mybir.dt.np(alloc.dtype)
            out_names.append(name)
            out_avals.append(jax.core.ShapedArray(shape, dtype))
            zero_outs.append(np.zeros(shape, dtype))
    n_params = len(in_names)
    n_outs = len(out_avals)
    in_names = in_names + out_names
    if partition_name is not None:
        in_names.append(partition_name)
    donate = tuple(range(n_params, n_params + n_outs))

    def _body(*args):
        operands = list(args)
        if partition_name is not None:
            operands.append(partition_id_tensor())
        outs = _bass_exec_p.bind(
            *operands,
            out_avals=tuple(out_avals),
            in_names=tuple(in_names),
            out_names=tuple(out_names),
            lowering_input_output_aliases=(),
            sim_require_finite=True,
            sim_require_nnan=True,
            nc=nc,
        )
        return tuple(outs)

    per_core = [[np.asarray(m[name]) for name in in_names[:n_params]]
                for m in in_maps]
    if n_cores == 1:
        out_arrs = jax.jit(_body, donate_argnums=donate, keep_unused=True)(
            *per_core[0], *zero_outs)
        return [{n: np.asarray(out_arrs[i]) for i, n in enumerate(out_names)}]

    mesh = Mesh(np.asarray(devices), ("core",))
    in_specs = (PartitionSpec("core"),) * (n_params + n_outs)
    out_specs = (PartitionSpec("core"),) * len(out_names)
    sharded = jax.jit(
        shard_map(_body, mesh=mesh, in_specs=in_specs, out_specs=out_specs,
                  check_rep=False),
        donate_argnums=donate, keep_unused=True)
    concat_in = [np.concatenate([per_core[c][i] for c in range(n_cores)],
                                axis=0) for i in range(n_params)]
    concat_zeros = [np.zeros((n_cores * z.shape[0], *z.shape[1:]), z.dtype)
                    for z in zero_outs]
    out_arrs = sharded(*concat_in, *concat_zeros)
    return [
        {n: np.asarray(out_arrs[i]).reshape(n_cores, *out_avals[i].shape)[c]
         for i, n in enumerate(out_names)}
        for c in range(n_cores)
    ]


_MODULES = {}
_WARM = set()


def _get_module(cls):
    if cls not in _MODULES:
        _MODULES[cls] = build_module(cls)
    return _MODULES[cls]


def kernel(x, W_in, b_in, W_out, b_out):
    import jax
    x = np.asarray(x, np.float32)
    W_in = np.asarray(W_in, np.float32)
    b_in = np.asarray(b_in, np.float32)
    W_out = np.asarray(W_out, np.float32)
    b_out = np.asarray(b_out, np.float32)

    late_maps, early_maps = make_in_maps(x, W_in, b_in, W_out)
    nc_late = _get_module("late")
    nc_early = _get_module("early")

    devs = jax.devices()
    results = {}
    errs = {}

    def run(tag, nc, maps, devices):
        try:
            results[tag] = _run_group(nc, maps, devices)
        except Exception as e:  # noqa: BLE001
            errs[tag] = e

    # first call per module compiles (serialize those); afterwards the two
    # device groups (cores 0-5 and 6-7) execute concurrently
    t1 = threading.Thread(target=run, args=("late", nc_late, late_maps, devs[0:6]))
    t2 = threading.Thread(target=run, args=("early", nc_early, early_maps, devs[6:8]))
    if not _WARM:
        t1.start(); t1.join()
        t2.start(); t2.join()
        _WARM.add(True)
    else:
        t1.start(); t2.start()
        t1.join(); t2.join()
    if errs:
        raise next(iter(errs.values()))

    return assemble_output(results["late"], results["early"], b_out)


# revision 14
# speedup vs baseline: 1.8936x; 1.8936x over previous
"""Multi-head causal attention (B=1, S=4096, D=768, H=12) on 8 trn2 NeuronCores.

Sharding: tensor-parallel over heads + causal-balanced split of the query range.
  - cores 0-5 ("late"):  2 heads each, q in [1792, 4096), k in [0, 4096)
  - cores 6-7 ("early"): 6 heads each, q in [0, 1792),  k in [0, 1792)

v2 design (vs baseline):
  - bf16 matmul operands everywhere (x, W, q, k, v, probs, Wo). Enables FWL
    (2x faster LDWEIGHTS) which unlocks true row-tiled concurrency of the two
    heads' score matmuls (head A on PE rows 0-63, head B on rows 64-127).
  - projection chunks (256 seq) interleaved with attention qtiles so the PE
    never idles long enough for the HAM clock-gate to re-throttle, and the
    scalar engine's exp stream (the real wall, ~0.85ns/elem/lane) overlaps
    the projection phase.
  - one exp instruction per (2-ktile x 2-head) group: [128, 1024] free.
  - both heads' flash accumulators packed into ONE psum bank [65 used, 512]
    (start=True only on the first matmul: the bank-wide has_written clear
    makes head B's first accumulate an overwrite; stop=True only on the
    final matmul).  Out-projection psum shares the same pool tag.
  - normalization: reciprocal_approx_fast + gpsimd partition_broadcast.
  - y written as bf16 partials, one DMA per qtile (host sums in fp32).

All inputs are taken at full shape; slicing/transposition happens on host.
"""

import sys
import threading

sys.path.insert(0, "/opt/trn_rl_repo")

import numpy as np
import ml_dtypes

import concourse.bass as bass
import concourse.mybir as mybir
import concourse.tile as tile
from concourse import bacc
from concourse.masks import make_identity

# ---------------------------------------------------------------- constants
B, S, D, H, DH = 1, 4096, 768, 12, 64
SCALE = DH ** -0.5
P = 128          # sbuf partitions
QT = 256         # query tile (free axis of scores)
KT = 128         # key tile (partition axis of scores)
CK = 256         # projection chunk (seq)
XW = 512         # x dma tile width (2 chunks)
GMAX = 3         # max ktiles per score/exp group
SPLIT = 1792     # early/late query split point
DT = mybir.dt.float32
BF = mybir.dt.bfloat16

CLASSES = {
    # name: (n_pairs, q0, q1, k_len)
    "late": (1, SPLIT, S, S),
    "early": (3, 0, SPLIT, SPLIT),
}


def _groups(n):
    """Split n (even) non-diagonal ktiles into chunks of 3 and 2."""
    out = []
    while n >= 5 or n == 3:
        out.append(3)
        n -= 3
    while n > 0:
        out.append(2)
        n -= 2
    return out


def build_module(cls):
    n_pairs, q0, q1, k_len = CLASSES[cls]
    f_c = P * n_pairs            # per-core feature width of each projection
    q_len = q1 - q0
    n_ck = k_len // CK           # projection chunks
    n_kt = k_len // KT           # ktiles of the core's k-support
    n_qt = q_len // QT           # qtiles of the core's q-range
    n_dt = D // P                # 6 contraction tiles for the projections
    c_q0 = q0 // CK              # first chunk whose q-projection is needed
    pre = (q0 + QT) // CK        # chunks needed before qtile 0 can run

    nc = bacc.Bacc("TRN2", target_bir_lowering=False, debug=False,
                   enable_asserts=True, num_devices=1)

    xT = nc.dram_tensor("xT", [D, k_len], BF, kind="ExternalInput")
    wqT = nc.dram_tensor("wqT", [D, f_c], BF, kind="ExternalInput")
    wkT = nc.dram_tensor("wkT", [D, f_c], BF, kind="ExternalInput")
    wvT = nc.dram_tensor("wvT", [D, f_c], BF, kind="ExternalInput")
    bq = nc.dram_tensor("bq", [f_c, 1], DT, kind="ExternalInput")
    bv = nc.dram_tensor("bv", [f_c, 1], DT, kind="ExternalInput")
    woT = nc.dram_tensor("woT", [f_c, D], BF, kind="ExternalInput")
    dmask = nc.dram_tensor("dmask", [P, 2 * QT], BF, kind="ExternalInput")
    yT = nc.dram_tensor("yT", [D, q_len], BF, kind="ExternalOutput")

    with tile.TileContext(nc) as tc:
        with (
            tc.tile_pool(name="w", bufs=1) as sb_w,
            tc.tile_pool(name="x", bufs=2) as sb_x,
            tc.tile_pool(name="persist", bufs=1) as sb_per,
            tc.tile_pool(name="vt", bufs=2) as sb_vt,
            tc.tile_pool(name="exp", bufs=3) as sb_exp,
            tc.tile_pool(name="aTp", bufs=2) as sb_a,
            tc.tile_pool(name="rn", bufs=3) as sb_rn,
            tc.tile_pool(name="yout", bufs=2) as sb_y,
            tc.tile_pool(name="big", bufs=2, space="PSUM") as ps_big,
            tc.tile_pool(name="av", bufs=1, space="PSUM") as ps_av,
        ):
            # ---------------- constants / weights to SBUF
            wq_sb = sb_w.tile([P, n_dt, f_c], BF, tag="wq")
            nc.sync.dma_start(out=wq_sb, in_=wqT.rearrange("(t p) f -> p t f", p=P))
            wk_sb = sb_w.tile([P, n_dt, f_c], BF, tag="wk")
            nc.sync.dma_start(out=wk_sb, in_=wkT.rearrange("(t p) f -> p t f", p=P))
            wv_sb = sb_w.tile([P, n_dt, f_c], BF, tag="wv")
            nc.sync.dma_start(out=wv_sb, in_=wvT.rearrange("(t p) f -> p t f", p=P))
            bq_sb = sb_w.tile([P, n_pairs], DT, tag="bq")
            nc.sync.dma_start(out=bq_sb, in_=bq.rearrange("(n p) o -> p (n o)", p=P))
            bv_sb = sb_w.tile([P, n_pairs], DT, tag="bv")
            nc.sync.dma_start(out=bv_sb, in_=bv.rearrange("(n p) o -> p (n o)", p=P))
            wo_sb = sb_w.tile([P, n_pairs, n_dt, P], BF, tag="wo")
            nc.sync.dma_start(
                out=wo_sb,
                in_=woT.rearrange("(n p) (t m) -> p n t m", p=P, m=P))
            dmask_sb = sb_w.tile([P, 2, QT], BF, tag="dmask")
            nc.sync.dma_start(
                out=dmask_sb, in_=dmask.rearrange("p (a q) -> p a q", a=2))
            ident_f = sb_w.tile([P, P], DT, tag="ident_f")
            make_identity(nc, ident_f)
            ident = sb_w.tile([P, P], BF, tag="ident")
            nc.vector.tensor_copy(ident, ident_f)

            # ---------------- persistent activations (head pair packed on
            # partitions: head A rows 0-63, head B rows 64-127)
            qT = [sb_per.tile([P, q_len], BF, tag=f"qT{p}", name=f"qT{p}")
                  for p in range(n_pairs)]
            kT = [sb_per.tile([P, k_len], BF, tag=f"kT{p}", name=f"kT{p}")
                  for p in range(n_pairs)]
            # per ktile: [V_A(64) | 1 | pad | V_B(64) | 1 | pad], k on partitions
            vkt = [sb_per.tile([P, n_kt, 132], BF, tag=f"vk{p}", name=f"vk{p}")
                   for p in range(n_pairs)]
            for p in range(n_pairs):
                nc.vector.memset(vkt[p][:, :, 64:65], 1.0)
                nc.vector.memset(vkt[p][:, :, 130:131], 1.0)

            # ---------------- projection chunk (CK=256 seq positions)
            cur_xts = []

            def emit_chunk(c):
                s0 = c * CK
                if c % 2 == 0:  # dma covers chunks c, c+1
                    w = min(XW, k_len - s0)
                    cur_xts.clear()
                    for dti in range(n_dt):
                        xt = sb_x.tile([P, XW], BF, tag=f"xt{dti}")
                        nc.sync.dma_start(
                            out=xt[:, :w],
                            in_=xT[dti * P:(dti + 1) * P, s0:s0 + w])
                        cur_xts.append(xt)
                xts = cur_xts
                xo = (c % 2) * CK
                do_q = c >= c_q0
                for p in range(n_pairs):
                    ps = ps_big.tile([P, 3, CK], DT, tag="big", name="ps_prj")
                    # each chain fully start->stop before the next begins: a
                    # start=True clears has_written for its WHOLE psum bank,
                    # so chains sharing a bank must not interleave
                    w_all = {0: wq_sb, 1: wk_sb, 2: wv_sb}
                    for i in ((0, 1, 2) if do_q else (1, 2)):
                        for dti in range(n_dt):
                            nc.tensor.matmul(
                                ps[:, i, :],
                                w_all[i][:, dti, p * P:(p + 1) * P],
                                xts[dti][:, xo:xo + CK],
                                start=dti == 0, stop=dti == n_dt - 1)
                    # k/v staging: scalar engine for the 3-pair class (its
                    # vector engine is busier), vector for the 1-pair class
                    # (its scalar engine is the exp wall)
                    if n_pairs > 1:
                        nc.scalar.copy(kT[p][:, s0:s0 + CK], ps[:, 1, :])
                    else:
                        nc.vector.tensor_copy(kT[p][:, s0:s0 + CK], ps[:, 1, :])
                    if do_q:
                        nc.vector.tensor_scalar_add(
                            qT[p][:, s0 - q0:s0 - q0 + CK],
                            ps[:, 0, :], bq_sb[:, p:p + 1])
                    vt = sb_vt.tile([P, CK], BF, tag=f"vt{p}")
                    if n_pairs > 1:
                        nc.scalar.activation(
                            vt, ps[:, 2, :],
                            mybir.ActivationFunctionType.Identity,
                            bias=bv_sb[:, p:p + 1])
                    else:
                        nc.vector.tensor_scalar_add(
                            vt, ps[:, 2, :], bv_sb[:, p:p + 1])
                    # transpose each 128-wide ktile into vkt layout
                    for j in range(CK // KT):
                        kt_i = (s0 // KT) + j
                        pt = ps_big.tile([P, P], BF, tag="big", name="pt")
                        nc.tensor.transpose(
                            pt, vt[:, j * KT:(j + 1) * KT], ident)
                        dst = vkt[p][:, kt_i, :].rearrange(
                            "p (h c) -> p h c", h=2)[:, :, 0:64]
                        nc.vector.tensor_copy(
                            dst, pt.rearrange("p (h c) -> p h c", h=2))

            # ---------------- attention qtile
            def emit_qtile(qt):
                g = q0 // QT + qt
                n_kt_q = 2 * g + 2
                plan = [(c, False) for c in _groups(n_kt_q - 2)] + [(2, True)]
                a_tiles = []
                for p in range(n_pairs):
                    # av: [P, (head, half, QT)] -- each head's accumulation
                    # group gets its own psum bank (numerator rows 0-63 +
                    # denominator row 64 in the first half); the second half
                    # of each bank doubles as out-projection psum
                    av = ps_av.tile([P, 2, 2, QT], DT, tag="av", name="av")
                    qh = [qT[p][hi * 64:(hi + 1) * 64,
                                qt * QT:(qt + 1) * QT] for hi in (0, 1)]
                    kt0 = 0
                    for (gsz, diag) in plan:
                        kts = list(range(kt0, kt0 + gsz))
                        kt0 += gsz
                        ps_sc = ps_big.tile([P, 2, GMAX, QT], DT, tag="big",
                                            name="ps_sc")
                        for j, k in enumerate(kts):
                            for hi in (0, 1):
                                nc.tensor.matmul(
                                    ps_sc[:, hi, j, :],
                                    kT[p][hi * 64:(hi + 1) * 64,
                                          k * KT:(k + 1) * KT],
                                    qh[hi], start=True, stop=True)
                        ex = sb_exp.tile([P, 2, GMAX, QT], BF, tag="ex")
                        nc.scalar.activation(
                            ex[:, :, 0:gsz, :], ps_sc[:, :, 0:gsz, :],
                            mybir.ActivationFunctionType.Exp, scale=SCALE)
                        if diag:
                            for hi in (0, 1):
                                nc.vector.tensor_mul(
                                    ex[:, hi, 0:2, :], ex[:, hi, 0:2, :],
                                    dmask_sb)
                        for j, k in enumerate(kts):
                            for hi in (0, 1):
                                nc.tensor.matmul(
                                    av[0:65, hi, 0, :],
                                    vkt[p][:, k, 66 * hi:66 * hi + 65],
                                    ex[:, hi, j, :],
                                    start=(k == 0),
                                    stop=(k == n_kt_q - 1))
                    # normalize: a = num * (1/den)
                    aT = sb_a.tile([P, QT], BF, tag=f"aT{p}")
                    # dens: psum row 64 of each head's bank -> one sbuf row;
                    # broadcast raw dens to all partitions (gpsimd), then a
                    # full-tile reciprocal_approx_fast (the custom DVE op is
                    # only correct on [128, N] base-0 tiles on HW)
                    dd = sb_rn.tile([1, 2, QT], DT, tag="dd")
                    nc.vector.tensor_copy(dd, av[64:65, :, 0, :])
                    db = sb_rn.tile([P, 2, QT], DT, tag="db")
                    nc.gpsimd.partition_broadcast(
                        db.rearrange("p h q -> p (h q)"),
                        dd.rearrange("p h q -> p (h q)"))
                    rb = sb_rn.tile([P, 2, QT], DT, tag="rb")
                    nc.vector.reciprocal_approx_fast(rb, db)
                    for hi in (0, 1):
                        nc.vector.tensor_mul(
                            aT[hi * 64:(hi + 1) * 64, :],
                            av[0:64, hi, 0, :], rb[hi * 64:hi * 64 + 64, hi, :])
                    a_tiles.append(aT)
                # out-projection (psum shares the "av" pool tag)
                ysb = sb_y.tile([P, n_dt, QT], BF, tag="y")
                for mt in range(n_dt):
                    ps_y = av[:, mt % 2, 1, :]
                    for p in range(n_pairs):
                        nc.tensor.matmul(
                            ps_y, wo_sb[:, p, mt, :], a_tiles[p],
                            start=(p == 0), stop=(p == n_pairs - 1))
                    nc.vector.tensor_copy(ysb[:, mt, :], ps_y)
                nc.sync.dma_start(
                    out=yT.rearrange("(t p) q -> p t q", p=P)[
                        :, :, qt * QT:(qt + 1) * QT],
                    in_=ysb)

            # ---------------- schedule: prefix chunks, then interleave
            for c in range(pre):
                emit_chunk(c)
            for qt in range(n_qt):
                emit_qtile(qt)
                if pre + qt < n_ck:
                    emit_chunk(pre + qt)

    nc.compile()
    return nc


# ---------------------------------------------------------------- host side
def _head_cols(heads):
    """column indices into a [*, 768] head-blocked axis for the given heads"""
    return np.concatenate([np.arange(h * DH, (h + 1) * DH) for h in heads])


def make_in_maps(x, W_in, b_in, W_out):
    """Returns (late_in_maps[6], early_in_maps[2])."""
    xT = np.ascontiguousarray(x.reshape(S, D).T).astype(ml_dtypes.bfloat16)
    WT = np.ascontiguousarray(W_in.T)                     # [768, 2304]
    WoT = np.ascontiguousarray(W_out.T)                   # [768, 768]

    tri = np.triu(np.ones((P, P), np.float32))            # k <= q
    dm = np.zeros((P, 2 * QT), np.float32)
    dm[:, 0:128] = tri          # diag ktile j=0: [tri | ones]
    dm[:, 128:256] = 1.0
    dm[:, 384:512] = tri        # diag ktile j=1: [zeros | tri]
    dm = dm.astype(ml_dtypes.bfloat16)

    def core_inputs(heads, cls):
        _, q0, q1, k_len = CLASSES[cls]
        cols = _head_cols(heads)
        bf = ml_dtypes.bfloat16
        wq = np.ascontiguousarray(WT[:, cols]).astype(bf)
        wk = np.ascontiguousarray(WT[:, 768 + cols]).astype(bf)
        wv = np.ascontiguousarray(WT[:, 1536 + cols]).astype(bf)
        bqc = np.ascontiguousarray(b_in[cols][:, None]).astype(np.float32)
        bvc = np.ascontiguousarray(
            b_in[1536 + cols][:, None]).astype(np.float32)
        wo = np.ascontiguousarray(WoT[cols, :]).astype(bf)
        return {
            "xT": np.ascontiguousarray(xT[:, :k_len]),
            "wqT": wq, "wkT": wk, "wvT": wv,
            "bq": bqc, "bv": bvc, "woT": wo, "dmask": dm,
        }

    late = [core_inputs([2 * c, 2 * c + 1], "late") for c in range(6)]
    early = [core_inputs(list(range(6 * e, 6 * e + 6)), "early")
             for e in range(2)]
    return late, early


def assemble_output(late_res, early_res, b_out):
    yT = np.zeros((D, S), np.float32)
    for r in late_res:
        yT[:, SPLIT:] += np.asarray(r["yT"], dtype=np.float32)
    for r in early_res:
        yT[:, :SPLIT] += np.asarray(r["yT"], dtype=np.float32)
    y = yT.T + b_out[None, :]
    return y.reshape(B, S, D).astype(np.float32)


# ------------------------------------------------- pjrt runner (explicit devices)
def _run_group(nc, in_maps, devices):
    """run_bass_via_pjrt equivalent on an explicit device subset."""
    import jax
    from jax.sharding import Mesh, PartitionSpec
    from jax.experimental.shard_map import shard_map
    from concourse import bass2jax
    from concourse.bass2jax import _bass_exec_p, partition_id_tensor

    bass2jax.install_neuronx_cc_hook()
    n_cores = len(in_maps)
    partition_name = (nc.partition_id_tensor.name
                      if nc.partition_id_tensor else None)

    in_names, out_names, out_avals, zero_outs = [], [], [], []
    for alloc in nc.m.functions[0].allocations:
        if not isinstance(alloc, mybir.MemoryLocationSet):
            continue
        name = alloc.memorylocations[0].name
        if alloc.kind == "ExternalInput":
            if name != partition_name:
                in_names.append(name)
        elif alloc.kind == "ExternalOutput":
            shape = tuple(alloc.tensor_shape)
            dtype = mybir.dt.np(alloc.dtype)
            out_names.append(name)
            out_avals.append(jax.core.ShapedArray(shape, dtype))
            zero_outs.append(np.zeros(shape, dtype))
    n_params = len(in_names)
    n_outs = len(out_avals)
    in_names = in_names + out_names
    if partition_name is not None:
        in_names.append(partition_name)
    donate = tuple(range(n_params, n_params + n_outs))

    def _body(*args):
        operands = list(args)
        if partition_name is not None:
            operands.append(partition_id_tensor())
        outs = _bass_exec_p.bind(
            *operands,
            out_avals=tuple(out_avals),
            in_names=tuple(in_names),
            out_names=tuple(out_names),
            lowering_input_output_aliases=(),
            sim_require_finite=True,
            sim_require_nnan=True,
            nc=nc,
        )
        return tuple(outs)

    per_core = [[np.asarray(m[name]) for name in in_names[:n_params]]
                for m in in_maps]
    if n_cores == 1:
        out_arrs = jax.jit(_body, donate_argnums=donate, keep_unused=True)(
            *per_core[0], *zero_outs)
        return [{n: np.asarray(out_arrs[i]) for i, n in enumerate(out_names)}]

    mesh = Mesh(np.asarray(devices), ("core",))
    in_specs = (PartitionSpec("core"),) * (n_params + n_outs)
    out_specs = (PartitionSpec("core"),) * len(out_names)
    sharded = jax.jit(
        shard_map(_body, mesh=mesh, in_specs=in_specs, out_specs=out_specs,
                  check_rep=False),
        donate_argnums=donate, keep_unused=True)
    concat_in = [np.concatenate([per_core[c][i] for c in range(n_cores)],
                                axis=0) for i in range(n_params)]
    concat_zeros = [np.zeros((n_cores * z.shape[0], *z.shape[1:]), z.dtype)
                    for z in zero_outs]
    out_arrs = sharded(*concat_in, *concat_zeros)
    return [
        {n: np.asarray(out_arrs[i]).reshape(n_cores, *out_avals[i].shape)[c]
         for i, n in enumerate(out_names)}
        for c in range(n_cores)
    ]


_MODULES = {}
_WARM = set()


def _get_module(cls):
    if cls not in _MODULES:
        _MODULES[cls] = build_module(cls)
    return _MODULES[cls]


def kernel(x, W_in, b_in, W_out, b_out):
    import jax
    x = np.asarray(x, np.float32)
    W_in = np.asarray(W_in, np.float32)
    b_in = np.asarray(b_in, np.float32)
    W_out = np.asarray(W_out, np.float32)
    b_out = np.asarray(b_out, np.float32)

    late_maps, early_maps = make_in_maps(x, W_in, b_in, W_out)
    nc_late = _get_module("late")
    nc_early = _get_module("early")

    devs = jax.devices()
    results = {}
    errs = {}

    def run(tag, nc, maps, devices):
        try:
            results[tag] = _run_group(nc, maps, devices)
        except Exception as e:  # noqa: BLE001
            errs[tag] = e

    # first call per module compiles (serialize those); afterwards the two
    # device groups (cores 0-5 and 6-7) execute concurrently
    t1 = threading.Thread(target=run, args=("late", nc_late, late_maps, devs[0:6]))
    t2 = threading.Thread(target=run, args=("early", nc_early, early_maps, devs[6:8]))
    if not _WARM:
        t1.start(); t1.join()
        t2.start(); t2.join()
        _WARM.add(True)
    else:
        t1.start(); t2.start()
        t1.join(); t2.join()
    if errs:
        raise next(iter(errs.values()))

    return assemble_output(results["late"], results["early"], b_out)


# revision 15
# speedup vs baseline: 2.0644x; 1.0902x over previous
"""Multi-head causal attention (B=1, S=4096, D=768, H=12) on 8 trn2 NeuronCores.

Sharding: tensor-parallel over heads + causal-balanced split of the query range.
  - cores 0-5 ("late"):  2 heads each, q in [1792, 4096), k in [0, 4096)
  - cores 6-7 ("early"): 6 heads each, q in [0, 1792),  k in [0, 1792)

v2 design (vs baseline):
  - bf16 matmul operands everywhere (x, W, q, k, v, probs, Wo). Enables FWL
    (2x faster LDWEIGHTS) which unlocks true row-tiled concurrency of the two
    heads' score matmuls (head A on PE rows 0-63, head B on rows 64-127).
  - projection chunks (256 seq) interleaved with attention qtiles so the PE
    never idles long enough for the HAM clock-gate to re-throttle, and the
    scalar engine's exp stream (the real wall, ~0.85ns/elem/lane) overlaps
    the projection phase.
  - one exp instruction per (2-ktile x 2-head) group: [128, 1024] free.
  - both heads' flash accumulators packed into ONE psum bank [65 used, 512]
    (start=True only on the first matmul: the bank-wide has_written clear
    makes head B's first accumulate an overwrite; stop=True only on the
    final matmul).  Out-projection psum shares the same pool tag.
  - normalization: reciprocal_approx_fast + gpsimd partition_broadcast.
  - y written as bf16 partials, one DMA per qtile (host sums in fp32).

All inputs are taken at full shape; slicing/transposition happens on host.
"""

import sys
import threading

sys.path.insert(0, "/opt/trn_rl_repo")

import numpy as np
import ml_dtypes

import concourse.bass as bass
import concourse.mybir as mybir
import concourse.tile as tile
from concourse import bacc
from concourse.masks import make_identity

# ---------------------------------------------------------------- constants
B, S, D, H, DH = 1, 4096, 768, 12, 64
SCALE = DH ** -0.5
P = 128          # sbuf partitions
QT = 256         # query tile (free axis of scores)
KT = 128         # key tile (partition axis of scores)
CK = 256         # projection chunk (seq)
XW = 512         # x dma tile width (2 chunks)
GMAX = 3         # max ktiles per score/exp group
SPLIT = 1792     # early/late query split point
DT = mybir.dt.float32
BF = mybir.dt.bfloat16

CLASSES = {
    # name: (n_pairs, q0, q1, k_len)
    "late": (1, SPLIT, S, S),
    "early": (3, 0, SPLIT, SPLIT),
}


def _groups(n):
    """Split n (even) non-diagonal ktiles into chunks of 3 and 2."""
    out = []
    while n >= 5 or n == 3:
        out.append(3)
        n -= 3
    while n > 0:
        out.append(2)
        n -= 2
    return out


def build_module(cls):
    n_pairs, q0, q1, k_len = CLASSES[cls]
    f_c = P * n_pairs            # per-core feature width of each projection
    q_len = q1 - q0
    n_ck = k_len // CK           # projection chunks
    n_kt = k_len // KT           # ktiles of the core's k-support
    n_qt = q_len // QT           # qtiles of the core's q-range
    n_dt = D // P                # 6 contraction tiles for the projections
    c_q0 = q0 // CK              # first chunk whose q-projection is needed
    pre = (q0 + QT) // CK        # chunks needed before qtile 0 can run

    nc = bacc.Bacc("TRN2", target_bir_lowering=False, debug=False,
                   enable_asserts=True, num_devices=1)

    xT = nc.dram_tensor("xT", [D, k_len], BF, kind="ExternalInput")
    wqT = nc.dram_tensor("wqT", [D, f_c], BF, kind="ExternalInput")
    wkT = nc.dram_tensor("wkT", [D, f_c], BF, kind="ExternalInput")
    wvT = nc.dram_tensor("wvT", [D, f_c], BF, kind="ExternalInput")
    bq = nc.dram_tensor("bq", [f_c, 1], DT, kind="ExternalInput")
    bv = nc.dram_tensor("bv", [f_c, 1], DT, kind="ExternalInput")
    woT = nc.dram_tensor("woT", [f_c, D], BF, kind="ExternalInput")
    dmask = nc.dram_tensor("dmask", [P, 2 * QT], BF, kind="ExternalInput")
    yT = nc.dram_tensor("yT", [D, q_len], BF, kind="ExternalOutput")

    with tile.TileContext(nc) as tc:
        with (
            tc.tile_pool(name="w", bufs=1) as sb_w,
            tc.tile_pool(name="x", bufs=2) as sb_x,
            tc.tile_pool(name="persist", bufs=1) as sb_per,
            tc.tile_pool(name="vt", bufs=2) as sb_vt,
            tc.tile_pool(name="exp", bufs=3) as sb_exp,
            tc.tile_pool(name="aTp", bufs=2) as sb_a,
            tc.tile_pool(name="rn", bufs=3) as sb_rn,
            tc.tile_pool(name="yout", bufs=2) as sb_y,
            tc.tile_pool(name="big", bufs=2, space="PSUM") as ps_big,
            tc.tile_pool(name="av", bufs=2, space="PSUM") as ps_av,
        ):
            # ---------------- constants / weights to SBUF
            wq_sb = sb_w.tile([P, n_dt, f_c], BF, tag="wq")
            nc.sync.dma_start(out=wq_sb, in_=wqT.rearrange("(t p) f -> p t f", p=P))
            wk_sb = sb_w.tile([P, n_dt, f_c], BF, tag="wk")
            nc.sync.dma_start(out=wk_sb, in_=wkT.rearrange("(t p) f -> p t f", p=P))
            wv_sb = sb_w.tile([P, n_dt, f_c], BF, tag="wv")
            nc.sync.dma_start(out=wv_sb, in_=wvT.rearrange("(t p) f -> p t f", p=P))
            bq_sb = sb_w.tile([P, n_pairs], DT, tag="bq")
            nc.sync.dma_start(out=bq_sb, in_=bq.rearrange("(n p) o -> p (n o)", p=P))
            bv_sb = sb_w.tile([P, n_pairs], DT, tag="bv")
            nc.sync.dma_start(out=bv_sb, in_=bv.rearrange("(n p) o -> p (n o)", p=P))
            wo_sb = sb_w.tile([P, n_pairs, n_dt, P], BF, tag="wo")
            nc.sync.dma_start(
                out=wo_sb,
                in_=woT.rearrange("(n p) (t m) -> p n t m", p=P, m=P))
            dmask_sb = sb_w.tile([P, 2, QT], BF, tag="dmask")
            nc.sync.dma_start(
                out=dmask_sb, in_=dmask.rearrange("p (a q) -> p a q", a=2))
            ident_f = sb_w.tile([P, P], DT, tag="ident_f")
            make_identity(nc, ident_f)
            ident = sb_w.tile([P, P], BF, tag="ident")
            nc.vector.tensor_copy(ident, ident_f)

            # ---------------- persistent activations (head pair packed on
            # partitions: head A rows 0-63, head B rows 64-127)
            qT = [sb_per.tile([P, q_len], BF, tag=f"qT{p}", name=f"qT{p}")
                  for p in range(n_pairs)]
            kT = [sb_per.tile([P, k_len], BF, tag=f"kT{p}", name=f"kT{p}")
                  for p in range(n_pairs)]
            # per ktile: [V_A(64) | 1 | pad | V_B(64) | 1 | pad], k on partitions
            vkt = [sb_per.tile([P, n_kt, 132], BF, tag=f"vk{p}", name=f"vk{p}")
                   for p in range(n_pairs)]
            for p in range(n_pairs):
                nc.vector.memset(vkt[p][:, :, 64:65], 1.0)
                nc.vector.memset(vkt[p][:, :, 130:131], 1.0)

            # ---------------- projection chunk (CK=256 seq positions)
            cur_xts = []

            def emit_chunk(c):
                s0 = c * CK
                if c % 2 == 0:  # dma covers chunks c, c+1
                    w = min(XW, k_len - s0)
                    cur_xts.clear()
                    for dti in range(n_dt):
                        xt = sb_x.tile([P, XW], BF, tag=f"xt{dti}")
                        nc.sync.dma_start(
                            out=xt[:, :w],
                            in_=xT[dti * P:(dti + 1) * P, s0:s0 + w])
                        cur_xts.append(xt)
                xts = cur_xts
                xo = (c % 2) * CK
                do_q = c >= c_q0
                for p in range(n_pairs):
                    ps = ps_big.tile([P, 3, CK], DT, tag="big", name="ps_prj")
                    # each chain fully start->stop before the next begins: a
                    # start=True clears has_written for its WHOLE psum bank,
                    # so chains sharing a bank must not interleave
                    w_all = {0: wq_sb, 1: wk_sb, 2: wv_sb}
                    for i in ((0, 1, 2) if do_q else (1, 2)):
                        for dti in range(n_dt):
                            nc.tensor.matmul(
                                ps[:, i, :],
                                w_all[i][:, dti, p * P:(p + 1) * P],
                                xts[dti][:, xo:xo + CK],
                                start=dti == 0, stop=dti == n_dt - 1)
                    # k/v staging: scalar engine for the 3-pair class (its
                    # vector engine is busier), vector for the 1-pair class
                    # (its scalar engine is the exp wall)
                    if n_pairs > 1:
                        nc.scalar.copy(kT[p][:, s0:s0 + CK], ps[:, 1, :])
                    else:
                        nc.vector.tensor_copy(kT[p][:, s0:s0 + CK], ps[:, 1, :])
                    if do_q:
                        nc.vector.tensor_scalar_add(
                            qT[p][:, s0 - q0:s0 - q0 + CK],
                            ps[:, 0, :], bq_sb[:, p:p + 1])
                    vt = sb_vt.tile([P, CK], BF, tag=f"vt{p}")
                    if n_pairs > 1:
                        nc.scalar.activation(
                            vt, ps[:, 2, :],
                            mybir.ActivationFunctionType.Identity,
                            bias=bv_sb[:, p:p + 1])
                    else:
                        nc.vector.tensor_scalar_add(
                            vt, ps[:, 2, :], bv_sb[:, p:p + 1])
                    # transpose each 128-wide ktile into vkt layout
                    for j in range(CK // KT):
                        kt_i = (s0 // KT) + j
                        pt = ps_big.tile([P, P], BF, tag="big", name="pt")
                        nc.tensor.transpose(
                            pt, vt[:, j * KT:(j + 1) * KT], ident)
                        dst = vkt[p][:, kt_i, :].rearrange(
                            "p (h c) -> p h c", h=2)[:, :, 0:64]
                        nc.vector.tensor_copy(
                            dst, pt.rearrange("p (h c) -> p h c", h=2))

            # ---------------- attention qtile
            def emit_qtile(qt):
                g = q0 // QT + qt
                n_kt_q = 2 * g + 2
                plan = [(c, False) for c in _groups(n_kt_q - 2)] + [(2, True)]
                a_tiles = []
                for p in range(n_pairs):
                    # av: one psum bank per (pair, qtile): head A numerator
                    # rows 0-63 + denominator row 64 in cols 0:256, head B in
                    # cols 256:512.  start=True only on the very first matmul
                    # (bank-wide has_written clear makes head B's first
                    # accumulate an overwrite), stop=True only on the last.
                    av = ps_av.tile([P, 2, QT], DT, tag="av", name="av")
                    qh = [qT[p][hi * 64:(hi + 1) * 64,
                                qt * QT:(qt + 1) * QT] for hi in (0, 1)]
                    kt0 = 0
                    for (gsz, diag) in plan:
                        kts = list(range(kt0, kt0 + gsz))
                        kt0 += gsz
                        ps_sc = ps_big.tile([P, 2, GMAX, QT], DT, tag="big",
                                            name="ps_sc")
                        for j, k in enumerate(kts):
                            for hi in (0, 1):
                                nc.tensor.matmul(
                                    ps_sc[:, hi, j, :],
                                    kT[p][hi * 64:(hi + 1) * 64,
                                          k * KT:(k + 1) * KT],
                                    qh[hi], start=True, stop=True)
                        ex = sb_exp.tile([P, 2, GMAX, QT], BF, tag="ex")
                        nc.scalar.activation(
                            ex[:, :, 0:gsz, :], ps_sc[:, :, 0:gsz, :],
                            mybir.ActivationFunctionType.Exp, scale=SCALE)
                        if diag:
                            for hi in (0, 1):
                                nc.vector.tensor_mul(
                                    ex[:, hi, 0:2, :], ex[:, hi, 0:2, :],
                                    dmask_sb)
                        for j, k in enumerate(kts):
                            for hi in (0, 1):
                                nc.tensor.matmul(
                                    av[0:65, hi, :],
                                    vkt[p][:, k, 66 * hi:66 * hi + 65],
                                    ex[:, hi, j, :],
                                    start=(k == 0 and hi == 0),
                                    stop=(k == n_kt_q - 1 and hi == 1))
                    # normalize: a = num * (1/den)
                    aT = sb_a.tile([P, QT], BF, tag=f"aT{p}")
                    # dens: psum row 64 of each head's bank -> one sbuf row;
                    # broadcast raw dens to all partitions (gpsimd), then a
                    # full-tile reciprocal_approx_fast (the custom DVE op is
                    # only correct on [128, N] base-0 tiles on HW)
                    dd = sb_rn.tile([1, 2 * QT], DT, tag="dd")
                    nc.vector.tensor_copy(dd, av[64:65, :, :].rearrange(
                        "p h q -> p (h q)"))
                    db = sb_rn.tile([P, 2 * QT], DT, tag="db")
                    nc.gpsimd.partition_broadcast(db, dd)
                    rb = sb_rn.tile([P, 2, QT], DT, tag="rb")
                    nc.vector.reciprocal_approx_fast(
                        rb.rearrange("p h q -> p (h q)"), db)
                    for hi in (0, 1):
                        nc.vector.tensor_mul(
                            aT[hi * 64:(hi + 1) * 64, :],
                            av[0:64, hi, :], rb[hi * 64:hi * 64 + 64, hi, :])
                    a_tiles.append(aT)
                # out-projection (psum shares the "av" pool tag)
                ysb = sb_y.tile([P, n_dt, QT], BF, tag="y")
                for mt in range(n_dt):
                    ps_y = ps_av.tile([P, 2, QT], DT, tag="av", name="ps_y")[:, 0, :]
                    for p in range(n_pairs):
                        nc.tensor.matmul(
                            ps_y, wo_sb[:, p, mt, :], a_tiles[p],
                            start=(p == 0), stop=(p == n_pairs - 1))
                    nc.vector.tensor_copy(ysb[:, mt, :], ps_y)
                nc.sync.dma_start(
                    out=yT.rearrange("(t p) q -> p t q", p=P)[
                        :, :, qt * QT:(qt + 1) * QT],
                    in_=ysb)

            # ---------------- schedule: prefix chunks, then interleave
            for c in range(pre):
                emit_chunk(c)
            for qt in range(n_qt):
                emit_qtile(qt)
                if pre + qt < n_ck:
                    emit_chunk(pre + qt)

    nc.compile()
    return nc


# ---------------------------------------------------------------- host side
def _head_cols(heads):
    """column indices into a [*, 768] head-blocked axis for the given heads"""
    return np.concatenate([np.arange(h * DH, (h + 1) * DH) for h in heads])


def make_in_maps(x, W_in, b_in, W_out):
    """Returns (late_in_maps[6], early_in_maps[2])."""
    xT = np.ascontiguousarray(x.reshape(S, D).T).astype(ml_dtypes.bfloat16)
    WT = np.ascontiguousarray(W_in.T)                     # [768, 2304]
    WoT = np.ascontiguousarray(W_out.T)                   # [768, 768]

    tri = np.triu(np.ones((P, P), np.float32))            # k <= q
    dm = np.zeros((P, 2 * QT), np.float32)
    dm[:, 0:128] = tri          # diag ktile j=0: [tri | ones]
    dm[:, 128:256] = 1.0
    dm[:, 384:512] = tri        # diag ktile j=1: [zeros | tri]
    dm = dm.astype(ml_dtypes.bfloat16)

    def core_inputs(heads, cls):
        _, q0, q1, k_len = CLASSES[cls]
        cols = _head_cols(heads)
        bf = ml_dtypes.bfloat16
        wq = np.ascontiguousarray(WT[:, cols]).astype(bf)
        wk = np.ascontiguousarray(WT[:, 768 + cols]).astype(bf)
        wv = np.ascontiguousarray(WT[:, 1536 + cols]).astype(bf)
        bqc = np.ascontiguousarray(b_in[cols][:, None]).astype(np.float32)
        bvc = np.ascontiguousarray(
            b_in[1536 + cols][:, None]).astype(np.float32)
        wo = np.ascontiguousarray(WoT[cols, :]).astype(bf)
        return {
            "xT": np.ascontiguousarray(xT[:, :k_len]),
            "wqT": wq, "wkT": wk, "wvT": wv,
            "bq": bqc, "bv": bvc, "woT": wo, "dmask": dm,
        }

    late = [core_inputs([2 * c, 2 * c + 1], "late") for c in range(6)]
    early = [core_inputs(list(range(6 * e, 6 * e + 6)), "early")
             for e in range(2)]
    return late, early


def assemble_output(late_res, early_res, b_out):
    yT = np.zeros((D, S), np.float32)
    for r in late_res:
        yT[:, SPLIT:] += np.asarray(r["yT"], dtype=np.float32)
    for r in early_res:
        yT[:, :SPLIT] += np.asarray(r["yT"], dtype=np.float32)
    y = yT.T + b_out[None, :]
    return y.reshape(B, S, D).astype(np.float32)


# ------------------------------------------------- pjrt runner (explicit devices)
def _run_group(nc, in_maps, devices):
    """run_bass_via_pjrt equivalent on an explicit device subset."""
    import jax
    from jax.sharding import Mesh, PartitionSpec
    from jax.experimental.shard_map import shard_map
    from concourse import bass2jax
    from concourse.bass2jax import _bass_exec_p, partition_id_tensor

    bass2jax.install_neuronx_cc_hook()
    n_cores = len(in_maps)
    partition_name = (nc.partition_id_tensor.name
                      if nc.partition_id_tensor else None)

    in_names, out_names, out_avals, zero_outs = [], [], [], []
    for alloc in nc.m.functions[0].allocations:
        if not isinstance(alloc, mybir.MemoryLocationSet):
            continue
        name = alloc.memorylocations[0].name
        if alloc.kind == "ExternalInput":
            if name != partition_name:
                in_names.append(name)
        elif alloc.kind == "ExternalOutput":
            shape = tuple(alloc.tensor_shape)
            dtype = mybir.dt.np(alloc.dtype)
            out_names.append(name)
            out_avals.append(jax.core.ShapedArray(shape, dtype))
            zero_outs.append(np.zeros(shape, dtype))
    n_params = len(in_names)
    n_outs = len(out_avals)
    in_names = in_names + out_names
    if partition_name is not None:
        in_names.append(partition_name)
    donate = tuple(range(n_params, n_params + n_outs))

    def _body(*args):
        operands = list(args)
        if partition_name is not None:
            operands.append(partition_id_tensor())
        outs = _bass_exec_p.bind(
            *operands,
            out_avals=tuple(out_avals),
            in_names=tuple(in_names),
            out_names=tuple(out_names),
            lowering_input_output_aliases=(),
            sim_require_finite=True,
            sim_require_nnan=True,
            nc=nc,
        )
        return tuple(outs)

    per_core = [[np.asarray(m[name]) for name in in_names[:n_params]]
                for m in in_maps]
    if n_cores == 1:
        out_arrs = jax.jit(_body, donate_argnums=donate, keep_unused=True)(
            *per_core[0], *zero_outs)
        return [{n: np.asarray(out_arrs[i]) for i, n in enumerate(out_names)}]

    mesh = Mesh(np.asarray(devices), ("core",))
    in_specs = (PartitionSpec("core"),) * (n_params + n_outs)
    out_specs = (PartitionSpec("core"),) * len(out_names)
    sharded = jax.jit(
        shard_map(_body, mesh=mesh, in_specs=in_specs, out_specs=out_specs,
                  check_rep=False),
        donate_argnums=donate, keep_unused=True)
    concat_in = [np.concatenate([per_core[c][i] for c in range(n_cores)],
                                axis=0) for i in range(n_params)]
    concat_zeros = [np.zeros((n_cores * z.shape[0], *z.shape[1:]), z.dtype)
                    for z in zero_outs]
    out_arrs = sharded(*concat_in, *concat_zeros)
    return [
        {n: np.asarray(out_arrs[i]).reshape(n_cores, *out_avals[i].shape)[c]
         for i, n in enumerate(out_names)}
        for c in range(n_cores)
    ]


_MODULES = {}
_WARM = set()


def _get_module(cls):
    if cls not in _MODULES:
        _MODULES[cls] = build_module(cls)
    return _MODULES[cls]


def kernel(x, W_in, b_in, W_out, b_out):
    import jax
    x = np.asarray(x, np.float32)
    W_in = np.asarray(W_in, np.float32)
    b_in = np.asarray(b_in, np.float32)
    W_out = np.asarray(W_out, np.float32)
    b_out = np.asarray(b_out, np.float32)

    late_maps, early_maps = make_in_maps(x, W_in, b_in, W_out)
    nc_late = _get_module("late")
    nc_early = _get_module("early")

    devs = jax.devices()
    results = {}
    errs = {}

    def run(tag, nc, maps, devices):
        try:
            results[tag] = _run_group(nc, maps, devices)
        except Exception as e:  # noqa: BLE001
            errs[tag] = e

    # first call per module compiles (serialize those); afterwards the two
    # device groups (cores 0-5 and 6-7) execute concurrently
    t1 = threading.Thread(target=run, args=("late", nc_late, late_maps, devs[0:6]))
    t2 = threading.Thread(target=run, args=("early", nc_early, early_maps, devs[6:8]))
    if not _WARM:
        t1.start(); t1.join()
        t2.start(); t2.join()
        _WARM.add(True)
    else:
        t1.start(); t2.start()
        t1.join(); t2.join()
    if errs:
        raise next(iter(errs.values()))

    return assemble_output(results["late"], results["early"], b_out)
